# revision 18
# baseline (speedup 1.0000x reference)
"""Causal multi-head attention (B=4, S=2048, D=1024, H=16) on 8 Trainium2 cores.

Sharding: core c handles batch b = c//2 and head-half hh = c%2 (8 heads, 512
head-dims). QKV/out projections are tensor-parallel over the head dim;
attention is embarrassingly parallel over (b, head). Out-projection partials
(rank-512) are summed pairwise on the host along with the output bias.

v3 vs the original baseline:
  - Q/K projections run in fp8e4m3 with DoubleRow (2 fp8 weights/cell,
    contraction 256/pass): ~2x fewer PE passes. fp8 elsewhere fails the
    2e-2 gate (exp/V/out paths are error-critical; measured via simulation),
    so V/scores/PV/out stay bf16.
  - K bias dropped entirely: softmax_k(q.(k+c)) == softmax_k(q.k), and the
    shared Q bias applied to K is likewise harmless, so one tensor_scalar_add
    biases the fused q|k tile.
  - V bias folded into the host-side output bias (softmax rows sum to 1).
  - SCALE folded into the exp activation's free scale parameter.
  - Output partials shipped bf16 (halves output DMA); host sums in fp32.
  - ScalarE does exp ONLY (the ~146us/core engine); mask multiplies moved to
    GpSimd; PSUM evacuations (qk bias, V copy, y copy, softmax norm) on DVE.
  - Emission is pipelined over query blocks: V-projection and q/k projection
    for block qb+1 and the out-projection for qb-1 fill the PE while exp
    drains, and output DMA overlaps the attention phase.
Attention layout is fully transposed (head-dim/d_model on partitions, seq on
free axis): S^T = K_h @ Q_h^T per 128-key block, head pairs packed into
disjoint 64-row PE groups; [V | ones].T @ expS^T emits both O^T and the
softmax denominator in one accumulation chain. Fully-masked key blocks are
skipped; diagonal blocks compute the valid column range plus one triangular
mask multiply.
"""

import numpy as np
import ml_dtypes

B, S, D = 4, 2048, 1024
H = 16
HH = 8          # heads per core
DK = 64
HD = 512        # head dims per core
N_CORES = 8
SCALE = DK ** -0.5
PB = 128        # partition block
QB = 512        # query block (matmul free dim)
NQB = S // QB   # 4
NKB = S // PB   # 16
KD = D // PB         # 8 bf16 d-tiles (V projection)
KDR = D // (2 * PB)  # 4 fp8 DoubleRow d-pair tiles (QK projection)
KO = HD // PB        # 4

_COMPILED = None
LAST_RESULTS = None


def _build():
    from contextlib import ExitStack
    import concourse.bass as bass
    import concourse.tile as tile
    from concourse import bacc, mybir

    BF16 = mybir.dt.bfloat16
    F32 = mybir.dt.float32
    F8 = mybir.dt.float8e4
    AF = mybir.ActivationFunctionType
    DR = mybir.MatmulPerfMode.DoubleRow

    nc = bacc.Bacc("TRN2", target_bir_lowering=False, debug=False,
                   num_devices=N_CORES)

    xT_d = nc.dram_tensor("xT", [D, S], BF16, kind="ExternalInput")
    wq_d = nc.dram_tensor("wq", [4 * PB, 2 * HD], F8, kind="ExternalInput")
    wk_d = nc.dram_tensor("wk", [4 * PB, 2 * HD], F8, kind="ExternalInput")
    wv_d = nc.dram_tensor("wv", [D, HD], BF16, kind="ExternalInput")
    wo_d = nc.dram_tensor("wo", [HD, D], BF16, kind="ExternalInput")
    bq_d = nc.dram_tensor("bq", [PB, KO], F32, kind="ExternalInput")
    mk_d = nc.dram_tensor("mk", [PB, 2 * PB], BF16, kind="ExternalInput")
    yT_d = nc.dram_tensor("yT", [D, S], BF16, kind="ExternalOutput")

    with tile.TileContext(nc) as tc, ExitStack() as ctx:
        persist = ctx.enter_context(tc.tile_pool(name="persist", bufs=1))
        work = ctx.enter_context(tc.tile_pool(name="work", bufs=8))
        nrm = ctx.enter_context(tc.tile_pool(name="nrm", bufs=6))
        yp = ctx.enter_context(tc.tile_pool(name="yp", bufs=4))
        psA = ctx.enter_context(tc.tile_pool(name="psA", bufs=3, space="PSUM"))
        psO = ctx.enter_context(tc.tile_pool(name="psO", bufs=2, space="PSUM"))

        xT = [persist.tile([PB, S], BF16, name=f"xT{k}") for k in range(KD)]
        xdr = [persist.tile([PB, 2, S], F8, name=f"x8{k}") for k in range(KDR)]
        wq = [persist.tile([PB, 2, HD], F8, name=f"wq{k}") for k in range(KDR)]
        wk = [persist.tile([PB, 2, HD], F8, name=f"wk{k}") for k in range(KDR)]
        wv = [persist.tile([PB, HD], BF16, name=f"wv{k}") for k in range(KD)]
        wo = [persist.tile([PB, D], BF16, name=f"wo{k}") for k in range(KO)]
        bq = persist.tile([PB, KO], F32)
        mk = persist.tile([PB, 2, PB], BF16)
        qk = [persist.tile([PB, 2, S], BF16, name=f"qk{k}") for k in range(KO)]
        # per key block: 8 heads x (64 ones cols | 64 V cols); ones first so
        # the PV matmul lands the softmax denominator on partitions 0..63
        # (reciprocal_approx_fast requires base partition 0)
        v = [persist.tile([PB, HH, 2 * DK], BF16, name=f"v{k}")
             for k in range(NKB)]
        onorm = [persist.tile([PB, S], BF16, name=f"on{k}") for k in range(KO)]

        # input DMAs: first x column-halves + wv first (the V-projection
        # prologue only touches key blocks 0..5 and query block 0)
        for k in range(KD):
            nc.sync.dma_start(xT[k][:, 0:S // 2],
                              xT_d[k * PB:(k + 1) * PB, 0:S // 2])
            nc.gpsimd.dma_start(wv[k][:], wv_d[k * PB:(k + 1) * PB, :])
        for k in range(KDR):
            nc.gpsimd.dma_start(wq[k][:],
                                wq_d[k * PB:(k + 1) * PB, :]
                                .rearrange("p (j f) -> p j f", j=2))
            nc.gpsimd.dma_start(wk[k][:],
                                wk_d[k * PB:(k + 1) * PB, :]
                                .rearrange("p (j f) -> p j f", j=2))
        for k2 in range(KO):
            nc.gpsimd.dma_start(wo[k2][:], wo_d[k2 * PB:(k2 + 1) * PB, :])
        nc.sync.dma_start(bq[:], bq_d[:])
        nc.sync.dma_start(mk[:], mk_d[:].rearrange("p (j c) -> p j c", j=2))
        for k in range(KD):
            nc.sync.dma_start(xT[k][:, S // 2:S],
                              xT_d[k * PB:(k + 1) * PB, S // 2:S])

        def xdr_chunk(qb, k2, j):
            qs = slice(qb * QB, (qb + 1) * QB)
            nc.vector.tensor_copy(xdr[k2][:, j, qs], xT[2 * k2 + j][:, qs])

        def v_proj(kbp):
            # key blocks 2*kbp, 2*kbp+1 (bf16; V feeds the output directly,
            # fp8 here fails the error budget)
            accv = psA.tile([PB, 2, QB], F32, tag="acc")
            for j in range(2):
                kb = 2 * kbp + j
                for k in range(KD):
                    nc.tensor.matmul(accv[:, j, :],
                                     xT[k][:, kb * PB:(kb + 1) * PB],
                                     wv[k][:], start=(k == 0),
                                     stop=(k == KD - 1))
            for j in range(2):
                kb = 2 * kbp + j
                nc.vector.tensor_copy(
                    v[kb][:, :, DK:2 * DK],
                    accv[:, j, :].rearrange("p (h d) -> p h d", h=HH))

        def qk_proj(m, qb):
            qs = slice(qb * QB, (qb + 1) * QB)
            ms = slice(m * PB, (m + 1) * PB)
            acc2 = psA.tile([PB, 2, QB], F32, tag="acc")
            for k in range(KDR):
                nc.tensor.matmul(acc2[:, 0, :], wq[k][:, :, ms],
                                 xdr[k][:, :, qs], start=(k == 0),
                                 stop=(k == KDR - 1), perf_mode=DR)
            for k in range(KDR):
                nc.tensor.matmul(acc2[:, 1, :], wk[k][:, :, ms],
                                 xdr[k][:, :, qs], start=(k == 0),
                                 stop=(k == KDR - 1), perf_mode=DR)
            # one biased evacuation for q AND k: a constant added to k
            # cancels in softmax, so sharing bq is harmless
            nc.vector.tensor_scalar_add(qk[m][:, :, qs], acc2[:],
                                        bq[:, m:m + 1])

        def out_proj(mo2, qb):
            qs = slice(qb * QB, (qb + 1) * QB)
            y_ps = psA.tile([PB, 2, QB], F32, tag="acc")
            for j in range(2):
                mo = 2 * mo2 + j
                for k2 in range(KO):
                    nc.tensor.matmul(y_ps[:, j, :],
                                     wo[k2][:, mo * PB:(mo + 1) * PB],
                                     onorm[k2][:, qs], start=(k2 == 0),
                                     stop=(k2 == KO - 1))
            y_sb = yp.tile([PB, 2, QB], BF16, tag="y")
            nc.vector.tensor_copy(y_sb[:], y_ps[:])
            nc.sync.dma_start(
                yT_d[2 * mo2 * PB:(2 * mo2 + 2) * PB, qs]
                .rearrange("(j p) q -> p j q", j=2), y_sb[:])

        def attention(hp, qb, fillers):
            # head pair 2*hp (score rows 0:64) + 2*hp+1 (rows 64:128)
            m = hp
            qs = slice(qb * QB, (qb + 1) * QB)
            nkb = 4 * qb + 4
            o_accs = [psO.tile([PB, QB], F32, tag="oacc", name=f"oacc{i}")
                      for i in range(2)]
            es = []
            for kb in range(nkb):
                t = kb - 4 * qb
                c0 = 0 if t < 0 else PB * t
                cs = slice(qb * QB + c0, (qb + 1) * QB)
                s_ps = psA.tile([PB, 2, QB], F32, tag="acc")
                for i, rb in enumerate((0, DK)):
                    nc.tensor.matmul(
                        s_ps[:, i, c0:QB],
                        qk[m][rb:rb + DK, 1, kb * PB:(kb + 1) * PB],
                        qk[m][rb:rb + DK, 0, cs], start=True, stop=True)
                e_sb = work.tile([PB, 2, QB], BF16, tag="e")
                nc.scalar.activation(e_sb[:, :, c0:QB], s_ps[:, :, c0:QB],
                                     AF.Exp, scale=SCALE)
                if t >= 0:
                    nc.gpsimd.tensor_mul(e_sb[:, :, c0:c0 + PB],
                                         e_sb[:, :, c0:c0 + PB], mk[:])
                es.append((e_sb, c0))
                # PV lags two blocks so exp+mask latency is off the PE
                # critical path even on filler-less iterations
                if kb >= 2:
                    _pv(hp, kb - 2, es[kb - 2], o_accs, False, nkb)
                if fillers and kb % 2 == 1 and kb < nkb - 2:
                    fillers.pop(0)()
            _pv(hp, nkb - 2, es[nkb - 2], o_accs, False, nkb)
            _pv(hp, nkb - 1, es[nkb - 1], o_accs, True, nkb)
            # norms first so DVE frees the o_acc banks while the deferred
            # fillers keep the PE busy
            for i in range(2):
                r_sb = nrm.tile([DK, QB], F32, tag="r")
                nc.vector.reciprocal_approx_fast(r_sb[:], o_accs[i][0:DK, :])
                nc.vector.tensor_mul(onorm[m][DK * i:DK * (i + 1), qs],
                                     o_accs[i][DK:2 * DK, :], r_sb[:])
            if fillers:
                fillers.pop(0)()

        def _pv(hp, kb, erec, o_accs, last, nkb):
            e_sb, c0 = erec
            for i in range(2):
                nc.tensor.matmul(o_accs[i][:, c0:QB],
                                 v[kb][:, 2 * hp + i, :],
                                 e_sb[:, i, c0:QB],
                                 start=(kb == 0), stop=last,
                                 skip_group_check=True)

        # prologue: V for the first 6 key blocks with the contraction
        # loop outermost, so each arriving xT d-tile immediately feeds the
        # PE instead of waiting for the full x DMA
        for k2 in range(KDR):
            for j in range(2):
                xdr_chunk(0, k2, j)
        for kb in range(4):
            nc.vector.memset(v[kb][:, :, 0:DK], 1.0)
        accvs = [psA.tile([PB, 2, QB], F32, tag="acc", name=f"accv{i}")
                 for i in range(3)]
        for k in range(KD):
            for kbp in range(3):
                for j in range(2):
                    kb = 2 * kbp + j
                    nc.tensor.matmul(accvs[kbp][:, j, :],
                                     xT[k][:, kb * PB:(kb + 1) * PB],
                                     wv[k][:], start=(k == 0),
                                     stop=(k == KD - 1))
        for kbp in range(3):
            for j in range(2):
                kb = 2 * kbp + j
                nc.vector.tensor_copy(
                    v[kb][:, :, DK:2 * DK],
                    accvs[kbp][:, j, :].rearrange("p (h d) -> p h d", h=HH))
        for m in range(KO):
            qk_proj(m, 0)
        for kb in range(4, NKB):
            nc.vector.memset(v[kb][:, :, 0:DK], 1.0)

        for qb in range(NQB):
            fillers = []
            if qb + 1 < NQB:
                for k2 in range(KDR):
                    for j in range(2):
                        fillers.append(
                            lambda qb=qb, k2=k2, j=j: xdr_chunk(qb + 1, k2, j))
                for kbp in range(max(3, 2 * (qb + 1)), 2 * (qb + 2)):
                    fillers.append(lambda kbp=kbp: v_proj(kbp))
                for m in range(KO):
                    fillers.append(lambda m=m, qb=qb: qk_proj(m, qb + 1))
            if qb >= 1:
                for mo2 in range(KO):
                    fillers.append(
                        lambda mo2=mo2, qb=qb: out_proj(mo2, qb - 1))
            for hp in range(KO):
                attention(hp, qb, fillers)
            for f in fillers:
                f()
        for mo2 in range(KO):
            out_proj(mo2, NQB - 1)

    nc.compile()
    return nc


def _get_compiled():
    global _COMPILED
    if _COMPILED is None:
        _COMPILED = _build()
    return _COMPILED


def _dr_rows(w):
    # [D_in, F] -> DoubleRow pair layout: row r = 128*(d//256) + d%128,
    # col block j = (d//128) % 2
    d_in, f = w.shape
    return np.ascontiguousarray(
        w.reshape(d_in // 256, 2, PB, f).transpose(0, 2, 1, 3)
        .reshape(d_in // 2, 2 * f))


def _make_in_maps(x, Wq, bq, Wk, Wv, Wo):
    bf16 = ml_dtypes.bfloat16
    f8 = ml_dtypes.float8_e4m3fn
    f32 = np.float32

    def to8(a):
        return np.clip(a, -240, 240).astype(f8)

    # inclusive lower-triangular mask for diagonal 128x128 blocks
    p_idx = np.arange(PB)[:, None]
    c_idx = np.arange(PB)[None, :]
    mk = np.tile((p_idx <= c_idx).astype(bf16), (1, 2))

    in_maps = []
    for c in range(N_CORES):
        b, hh = c // 2, c % 2
        cs = slice(hh * HD, (hh + 1) * HD)
        xT = np.ascontiguousarray(x[b].T)
        in_maps.append({
            "xT": xT.astype(bf16),
            "wq": to8(_dr_rows(Wq[:, cs])),
            "wk": to8(_dr_rows(Wk[:, cs])),
            "wv": np.ascontiguousarray(Wv[:, cs]).astype(bf16),
            "wo": np.ascontiguousarray(Wo[cs, :]).astype(bf16),
            "bq": np.ascontiguousarray(
                bq[cs].astype(f32).reshape(KO, PB).T),
            "mk": mk,
        })
    return in_maps


def _reference_fallback(x, mask, Wq, bq, Wk, bk, Wv, bv, Wo, bo):
    out = np.empty((B, S, D), dtype=np.float32)
    for b in range(B):
        q = (x[b] @ Wq + bq).reshape(S, H, DK).transpose(1, 0, 2)
        k = (x[b] @ Wk + bk).reshape(S, H, DK).transpose(1, 0, 2)
        vv = (x[b] @ Wv + bv).reshape(S, H, DK).transpose(1, 0, 2)
        o = np.empty((H, S, DK), dtype=np.float32)
        for hi in range(H):
            s = (q[hi] @ k[hi].T) * SCALE
            s = np.where(mask[b], -1e9, s)
            s = s - s.max(axis=-1, keepdims=True)
            e = np.exp(s)
            p = e / e.sum(axis=-1, keepdims=True)
            o[hi] = p @ vv[hi]
        out[b] = o.transpose(1, 0, 2).reshape(S, D) @ Wo + bo
    return out


def kernel(x, mask, Wq, bq, Wk, bk, Wv, bv, Wo, bo, **kwargs):
    global LAST_RESULTS
    import os

    x = np.asarray(x, dtype=np.float32)
    mask = np.asarray(mask)

    causal = np.triu(np.ones((S, S), dtype=bool), k=1)
    if not all(np.array_equal(mask[b], causal) for b in range(B)):
        return _reference_fallback(np.asarray(x), mask, np.asarray(Wq),
                                   np.asarray(bq), np.asarray(Wk),
                                   np.asarray(bk), np.asarray(Wv),
                                   np.asarray(bv), np.asarray(Wo),
                                   np.asarray(bo))

    from concourse.bass_utils import run_bass_kernel_spmd

    nc = _get_compiled()
    Wq, bq, Wk = np.asarray(Wq), np.asarray(bq), np.asarray(Wk)
    Wv, bv, Wo = np.asarray(Wv), np.asarray(bv), np.asarray(Wo)
    in_maps = _make_in_maps(x, Wq, bq, Wk, Wv, Wo)
    trace = bool(int(os.environ.get("KERNEL_PROFILE", "0")))
    res = run_bass_kernel_spmd(nc, in_maps, list(range(N_CORES)), trace=trace)
    LAST_RESULTS = res

    # K bias cancels in softmax; V bias folds into the output bias since
    # softmax rows sum to 1: y += bv @ Wo + bo
    bo_eff = (np.asarray(bv, dtype=np.float32) @ Wo.astype(np.float32)
              + np.asarray(bo, dtype=np.float32))
    out = np.empty((B, S, D), dtype=np.float32)
    for b in range(B):
        acc = (res.results[2 * b]["yT"].astype(np.float32)
               + res.results[2 * b + 1]["yT"].astype(np.float32))
        out[b] = acc.T + bo_eff
    return out
